# revision 10
# baseline (speedup 1.0000x reference)
"""Trainium2 Bass kernel for CustomConv2d:
  x [16, 32, 512, 512] f32, weight [32, 32, 3, 3] f32, bias [32] f32
  -> out [16, 32, 510, 510] f32   (stride 1, VALID padding, + bias)

Data-parallel over batch: 2 images per core across 8 NeuronCores.

v5 design (bf16 I/O + 4-tile [64,64] PE, tile-pure chains):
 - Host converts x/weight to bf16 and relayouts x into the exact SBUF strip
   layout, so every input DMA is one fully-contiguous 2MB transfer per strip
   tile. Output is dumped as packed [128, 510] bf16 drain tiles and
   unshuffled + upcast to f32 on the host.
 - SBUF x layout: strip s holds input rows 32s..32s+31; partition
   64*img + 32*(r%2) + ci, free offset 512*((r%32)//2) + w. One [128, 8192]
   bf16 tile per strip covers both images of the core.
 - Compute: per output row pair (y, y+1), y even: one 6-MM accumulation
   chain of [K=64, M=64] matmuls -- 3 kw taps x 2 two-row input windows,
   with 3-of-4 weight blocks useful per MM (75%, the ceiling for 2-row
   windows). Chains are tile-pure and banks row-half-pure: HW crashes if an
   accumulation group spans PE row tiles or a PSUM bank is written by more
   than one row quadrant (verified by probing), and the PE sustains only ~8
   concurrent matmul streams, so 4 tiles of [64,64] keep the whole array
   busy with big, cheap-to-issue matmuls (3072 total; LDWEIGHTS has ~90ns
   fixed cost and ~2.6x concurrency, so 18k+ small matmuls are issue-bound).
 - Blocks of 4 output rows (2 pairs): pair cp -> PSUM bank (il) partitions
   64cp+32h+co = row y0+2cp+h. 2 banks per block, bufs=3 -> 6 of 8 banks.
 - Drains are full [128, 510] bias-adds psum->bf16, alternating ScalarE
   (img0) / VectorE (img1). Row block 127 is y0=506 (recomputes rows
   506/507) so all drains stay uniform; host takes rows 508/509 from it.
"""
import numpy as np
from ml_dtypes import bfloat16

import concourse.bass as bass
import concourse.tile as tile
from concourse import bacc, mybir
from concourse.bass_utils import run_bass_kernel_spmd
from contextlib import ExitStack

F32 = mybir.dt.float32
BF16 = mybir.dt.bfloat16

N_FULL, C, H, W = 16, 32, 512, 512
HO = WO = 510
N_CORES = 8
N_PER = N_FULL // N_CORES          # 2 images per core
N_STRIPS = H // 32                 # 16 strips of 32 input rows
N_BLOCKS = 128                     # 4-output-row blocks (block 127: y0=506)


def _block_y0(mb):
    return 4 * mb if mb < N_BLOCKS - 1 else 506


def _build():
    nc = bacc.Bacc("TRN2", target_bir_lowering=False, debug=False, num_devices=1)
    x_d = nc.dram_tensor("x", [N_STRIPS, 128, 8192], BF16,
                         kind="ExternalInput").ap()
    w_d = nc.dram_tensor("w", [128, 384], BF16, kind="ExternalInput").ap()
    b_d = nc.dram_tensor("b", [128, 1], F32, kind="ExternalInput").ap()
    o_d = nc.dram_tensor("out", [N_BLOCKS, 128, 2 * WO], BF16,
                         kind="ExternalOutput").ap()

    with tile.TileContext(nc) as tc, ExitStack() as ctx:
        const_pool = ctx.enter_context(tc.tile_pool(name="const", bufs=1))
        x_pool = ctx.enter_context(tc.tile_pool(name="xs", bufs=3))
        ps_pool = ctx.enter_context(tc.tile_pool(name="ps", bufs=4, space="PSUM"))
        o_pool = ctx.enter_context(tc.tile_pool(name="ob", bufs=3))

        wv = const_pool.tile([128, 384], BF16)
        nc.sync.dma_start(wv[:], w_d[:])
        bt = const_pool.tile([128, 1], F32)
        nc.sync.dma_start(bt[:], b_d[:])

        xtiles = {}
        obtiles = {}

        def load_strip(s):
            xa = x_pool.tile([128, 8192], BF16, tag="x", name=f"xs_{s}")
            nc.scalar.dma_start(xa[:], x_d[s])
            xtiles[s] = xa

        def emit_block(mb):
            y0 = _block_y0(mb)
            banks = {}
            for il in range(N_PER):
                banks[il] = ps_pool.tile([128, 512], F32, tag=f"ps{il}",
                                         name=f"ps{il}_{mb}")
            for step in range(6):
                w, kw = divmod(step, 3)
                for il in range(N_PER):
                    for cp in range(2):
                        rw = y0 + 2 * cp + 2 * w       # window rows rw, rw+1
                        st, lrw = divmod(rw, 32)
                        t = lrw // 2
                        xa = xtiles[st]
                        nc.tensor.matmul(
                            banks[il][64 * cp:64 * cp + 64, 0:WO],
                            wv[64 * il:64 * il + 64,
                               64 * (3 * w + kw):64 * (3 * w + kw) + 64],
                            xa[64 * il:64 * il + 64, 512 * t + kw:512 * t + kw + WO],
                            start=(step == 0), stop=(step == 5),
                            skip_group_check=True,
                            tile_position=(64 * il, 64 * cp),
                        )
            # two consecutive blocks share one [128, 1020] output tile per
            # image -> 2040B output descriptors and half the DMA issues,
            # alternating the sync HWDGE and gpsimd SWDGE queues.
            pb, half = divmod(mb, 2)
            for il in range(N_PER):
                if half == 0:
                    obtiles[il] = o_pool.tile([128, 2 * WO], BF16,
                                              tag=f"ob{il}", name=f"ob{il}_{pb}")
                ob = obtiles[il]
                src = banks[il][0:128, 0:WO]
                dst = ob[:, half * WO:half * WO + WO]
                if il == 0:
                    nc.scalar.activation(
                        dst, src, mybir.ActivationFunctionType.Identity,
                        bias=bt[:])
                else:
                    nc.vector.tensor_scalar_add(dst, src, bt[:])
                if half == 1:
                    eng = nc.sync if (pb + il) % 2 == 0 else nc.gpsimd
                    eng.dma_start(o_d[2 * pb + il], ob[:])

        load_strip(0)
        for s in range(N_STRIPS):
            if s + 1 < N_STRIPS:
                load_strip(s + 1)
            for mb in range(8 * s, min(8 * s + 8, N_BLOCKS)):
                emit_block(mb)

    nc.compile()
    return nc


def _prep_inputs(x, weight, bias):
    """Host-side shard + relayout. Returns per-core in_maps."""
    x = np.asarray(x, dtype=np.float32)
    weight = np.asarray(weight, dtype=np.float32)
    bias = np.asarray(bias, dtype=np.float32)

    # x[2i+il, ci, 32s+2t+q, w] -> xs[i, s, 64*il+32*q+ci, 512*t+w]
    xr = x.reshape(N_CORES, N_PER, C, N_STRIPS, 16, 2, W)
    xr = xr.transpose(0, 3, 1, 5, 2, 4, 6)          # core, s, il, q, ci, t, w
    xs = np.ascontiguousarray(xr).reshape(N_CORES, N_STRIPS, 128, 8192)
    xs = xs.astype(bfloat16)

    # wv[64il + 32q + ci, 64*(3w+kw) + 32h + co] = weight[co, ci, 2w+q-h, kw]
    # (zero when kh = 2w+q-h is outside [0, 3))
    wk = np.zeros((2, 32, 6, 2, 32), dtype=np.float32)  # q, ci, (w,kw), h, co
    for w in range(2):
        for kw in range(3):
            for q in range(2):
                for h in range(2):
                    kh = 2 * w + q - h
                    if 0 <= kh <= 2:
                        wk[q, :, 3 * w + kw, h, :] = weight[:, :, kh, kw].T
    wv = wk.transpose(0, 1, 2, 3, 4).reshape(64, 384)
    wv = np.tile(wv, (2, 1)).astype(bfloat16)
    bt = np.tile(bias, 4)[:, None].astype(np.float32)

    return [{"x": xs[i], "w": wv, "b": bt} for i in range(N_CORES)]


def _unpack_output(results):
    """results: list of 8 dicts with 'out' [128, 128, 1020] bf16."""
    dev = np.stack([r["out"] for r in results], axis=0)
    # [core, pb, il, cp, h, co, half, w]; row = 8pb + 4half + 2cp + h
    dev = dev.reshape(N_CORES, N_BLOCKS // 2, N_PER, 2, 2, C, 2, WO)
    full = dev.transpose(0, 2, 5, 1, 6, 3, 4, 7)
    full = full.reshape(N_CORES, N_PER, C, 512, WO)
    out = np.empty((N_FULL, C, HO, WO), dtype=np.float32)
    o = out.reshape(N_CORES, N_PER, C, HO, WO)
    o[:, :, :, :508, :] = full[:, :, :, :508, :]
    # block 127 (pb=63, half=1) holds rows 506 + 2cp + h -> composed rows
    # 510/511 are true rows 508/509
    o[:, :, :, 508:510, :] = full[:, :, :, 510:512, :]
    return out


_NC = None


def kernel(x, weight, bias):
    global _NC
    if _NC is None:
        _NC = _build()
    in_maps = _prep_inputs(x, weight, bias)
    res = run_bass_kernel_spmd(_NC, in_maps, core_ids=list(range(N_CORES)))
    return _unpack_output(res.results)


# revision 15
# speedup vs baseline: 1.4080x; 1.4080x over previous
"""Trainium2 Bass kernel for CustomConv2d:
  x [16, 32, 512, 512] f32, weight [32, 32, 3, 3] f32, bias [32] f32
  -> out [16, 32, 510, 510] f32   (stride 1, VALID padding, + bias)

Data-parallel over batch: 2 images per core across 8 NeuronCores.

v5 design (bf16 I/O + 4-tile [64,64] PE, tile-pure chains):
 - Host converts x/weight to bf16 and relayouts x into the exact SBUF strip
   layout, so every input DMA is one fully-contiguous 2MB transfer per strip
   tile. Output is dumped as packed [128, 510] bf16 drain tiles and
   unshuffled + upcast to f32 on the host.
 - SBUF x layout: strip s holds input rows 32s..32s+31; partition
   64*img + 32*(r%2) + ci, free offset 512*((r%32)//2) + w. One [128, 8192]
   bf16 tile per strip covers both images of the core.
 - Compute: per output row pair (y, y+1), y even: one 6-MM accumulation
   chain of [K=64, M=64] matmuls -- 3 kw taps x 2 two-row input windows,
   with 3-of-4 weight blocks useful per MM (75%, the ceiling for 2-row
   windows). Chains are tile-pure and banks row-half-pure: HW crashes if an
   accumulation group spans PE row tiles or a PSUM bank is written by more
   than one row quadrant (verified by probing), and the PE sustains only ~8
   concurrent matmul streams, so 4 tiles of [64,64] keep the whole array
   busy with big, cheap-to-issue matmuls (3072 total; LDWEIGHTS has ~90ns
   fixed cost and ~2.6x concurrency, so 18k+ small matmuls are issue-bound).
 - Blocks of 4 output rows (2 pairs): pair cp -> PSUM bank (il) partitions
   64cp+32h+co = row y0+2cp+h. 2 banks per block, bufs=3 -> 6 of 8 banks.
 - Drains are full [128, 510] bias-adds psum->bf16, alternating ScalarE
   (img0) / VectorE (img1). Row block 127 is y0=506 (recomputes rows
   506/507) so all drains stay uniform; host takes rows 508/509 from it.
"""
import numpy as np
from ml_dtypes import bfloat16

import concourse.bass as bass
import concourse.tile as tile
from concourse import bacc, mybir
from concourse.bass_utils import run_bass_kernel_spmd
from contextlib import ExitStack

F32 = mybir.dt.float32
BF16 = mybir.dt.bfloat16

N_FULL, C, H, W = 16, 32, 512, 512
HO = WO = 510
N_CORES = 8
N_PER = N_FULL // N_CORES          # 2 images per core
N_STRIPS = H // 32                 # 16 strips of 32 input rows
N_BLOCKS = 128                     # 4-output-row blocks (block 127: y0=506)


def _block_y0(mb):
    return 4 * mb if mb < N_BLOCKS - 1 else 506


def _build():
    nc = bacc.Bacc("TRN2", target_bir_lowering=False, debug=False, num_devices=1)
    x_d = nc.dram_tensor("x", [N_STRIPS, 128, 8192], BF16,
                         kind="ExternalInput").ap()
    w_d = nc.dram_tensor("w", [128, 384], BF16, kind="ExternalInput").ap()
    b_d = nc.dram_tensor("b", [128, 1], F32, kind="ExternalInput").ap()
    o_d = nc.dram_tensor("out", [N_BLOCKS, 128, 2 * WO], BF16,
                         kind="ExternalOutput").ap()

    with tile.TileContext(nc) as tc, ExitStack() as ctx:
        const_pool = ctx.enter_context(tc.tile_pool(name="const", bufs=1))
        x_pool = ctx.enter_context(tc.tile_pool(name="xs", bufs=4))
        ps_pool = ctx.enter_context(tc.tile_pool(name="ps", bufs=4, space="PSUM"))
        o_pool = ctx.enter_context(tc.tile_pool(name="ob", bufs=3))

        wv = const_pool.tile([128, 384], BF16)
        nc.sync.dma_start(wv[:], w_d[:])
        bt = const_pool.tile([128, 1], F32)
        nc.sync.dma_start(bt[:], b_d[:])

        xtiles = {}
        obtiles = {}

        def load_strip(s):
            xa = x_pool.tile([128, 8192], BF16, tag="x", name=f"xs_{s}")
            nc.scalar.dma_start(xa[:], x_d[s])
            xtiles[s] = xa

        def emit_block(mb):
            y0 = _block_y0(mb)
            banks = {}
            for il in range(N_PER):
                banks[il] = ps_pool.tile([128, 512], F32, tag=f"ps{il}",
                                         name=f"ps{il}_{mb}")
            for step in range(6):
                w, kw = divmod(step, 3)
                for il in range(N_PER):
                    for cp in range(2):
                        rw = y0 + 2 * cp + 2 * w       # window rows rw, rw+1
                        st, lrw = divmod(rw, 32)
                        t = lrw // 2
                        xa = xtiles[st]
                        nc.tensor.matmul(
                            banks[il][64 * cp:64 * cp + 64, 0:WO],
                            wv[64 * il:64 * il + 64,
                               64 * (3 * w + kw):64 * (3 * w + kw) + 64],
                            xa[64 * il:64 * il + 64, 512 * t + kw:512 * t + kw + WO],
                            start=(step == 0), stop=(step == 5),
                            skip_group_check=True,
                            tile_position=(64 * il, 64 * cp),
                        )
            # two consecutive blocks share one [128, 1020] output tile per
            # image -> 2040B output descriptors and half the DMA issues,
            # alternating the sync HWDGE and gpsimd SWDGE queues.
            pb, half = divmod(mb, 2)
            for il in range(N_PER):
                if half == 0:
                    obtiles[il] = o_pool.tile([128, 2 * WO], BF16,
                                              tag=f"ob{il}", name=f"ob{il}_{pb}")
                ob = obtiles[il]
                src = banks[il][0:128, 0:WO]
                dst = ob[:, half * WO:half * WO + WO]
                if il == 0:
                    nc.scalar.activation(
                        dst, src, mybir.ActivationFunctionType.Identity,
                        bias=bt[:])
                else:
                    nc.vector.tensor_scalar_add(dst, src, bt[:])
                if half == 1:
                    nc.sync.dma_start(o_d[2 * pb + il], ob[:])

        load_strip(0)
        load_strip(1)
        for s in range(N_STRIPS):
            if s + 2 < N_STRIPS:
                load_strip(s + 2)
            for mb in range(8 * s, min(8 * s + 8, N_BLOCKS)):
                emit_block(mb)

    nc.compile()
    return nc


def _prep_inputs(x, weight, bias):
    """Host-side shard + relayout. Returns per-core in_maps."""
    x = np.asarray(x, dtype=np.float32)
    weight = np.asarray(weight, dtype=np.float32)
    bias = np.asarray(bias, dtype=np.float32)

    # x[2i+il, ci, 32s+2t+q, w] -> xs[i, s, 64*il+32*q+ci, 512*t+w]
    xr = x.reshape(N_CORES, N_PER, C, N_STRIPS, 16, 2, W)
    xr = xr.transpose(0, 3, 1, 5, 2, 4, 6)          # core, s, il, q, ci, t, w
    xs = np.ascontiguousarray(xr).reshape(N_CORES, N_STRIPS, 128, 8192)
    xs = xs.astype(bfloat16)

    # wv[64il + 32q + ci, 64*(3w+kw) + 32h + co] = weight[co, ci, 2w+q-h, kw]
    # (zero when kh = 2w+q-h is outside [0, 3))
    wk = np.zeros((2, 32, 6, 2, 32), dtype=np.float32)  # q, ci, (w,kw), h, co
    for w in range(2):
        for kw in range(3):
            for q in range(2):
                for h in range(2):
                    kh = 2 * w + q - h
                    if 0 <= kh <= 2:
                        wk[q, :, 3 * w + kw, h, :] = weight[:, :, kh, kw].T
    wv = wk.transpose(0, 1, 2, 3, 4).reshape(64, 384)
    wv = np.tile(wv, (2, 1)).astype(bfloat16)
    bt = np.tile(bias, 4)[:, None].astype(np.float32)

    return [{"x": xs[i], "w": wv, "b": bt} for i in range(N_CORES)]


def _unpack_output(results):
    """results: list of 8 dicts with 'out' [128, 128, 1020] bf16."""
    dev = np.stack([r["out"] for r in results], axis=0)
    # [core, pb, il, cp, h, co, half, w]; row = 8pb + 4half + 2cp + h
    dev = dev.reshape(N_CORES, N_BLOCKS // 2, N_PER, 2, 2, C, 2, WO)
    full = dev.transpose(0, 2, 5, 1, 6, 3, 4, 7)
    full = full.reshape(N_CORES, N_PER, C, 512, WO)
    out = np.empty((N_FULL, C, HO, WO), dtype=np.float32)
    o = out.reshape(N_CORES, N_PER, C, HO, WO)
    o[:, :, :, :508, :] = full[:, :, :, :508, :]
    # block 127 (pb=63, half=1) holds rows 506 + 2cp + h -> composed rows
    # 510/511 are true rows 508/509
    o[:, :, :, 508:510, :] = full[:, :, :, 510:512, :]
    return out


_NC = None


def kernel(x, weight, bias):
    global _NC
    if _NC is None:
        _NC = _build()
    in_maps = _prep_inputs(x, weight, bias)
    res = run_bass_kernel_spmd(_NC, in_maps, core_ids=list(range(N_CORES)))
    return _unpack_output(res.results)


# revision 22
# speedup vs baseline: 1.4245x; 1.0117x over previous
"""Trainium2 Bass kernel for CustomConv2d:
  x [16, 32, 512, 512] f32, weight [32, 32, 3, 3] f32, bias [32] f32
  -> out [16, 32, 510, 510] f32   (stride 1, VALID padding, + bias)

Data-parallel over batch: 2 images per core across 8 NeuronCores.

v5 design (bf16 I/O + 4-tile [64,64] PE, tile-pure chains):
 - Host converts x/weight to bf16 and relayouts x into the exact SBUF strip
   layout, so every input DMA is one fully-contiguous 2MB transfer per strip
   tile. Output is dumped as packed [128, 510] bf16 drain tiles and
   unshuffled + upcast to f32 on the host.
 - SBUF x layout: strip s holds input rows 32s..32s+31; partition
   64*img + 32*(r%2) + ci, free offset 512*((r%32)//2) + w. One [128, 8192]
   bf16 tile per strip covers both images of the core.
 - Compute: per output row pair (y, y+1), y even: one 6-MM accumulation
   chain of [K=64, M=64] matmuls -- 3 kw taps x 2 two-row input windows,
   with 3-of-4 weight blocks useful per MM (75%, the ceiling for 2-row
   windows). Chains are tile-pure and banks row-half-pure: HW crashes if an
   accumulation group spans PE row tiles or a PSUM bank is written by more
   than one row quadrant (verified by probing), and the PE sustains only ~8
   concurrent matmul streams, so 4 tiles of [64,64] keep the whole array
   busy with big, cheap-to-issue matmuls (3072 total; LDWEIGHTS has ~90ns
   fixed cost and ~2.6x concurrency, so 18k+ small matmuls are issue-bound).
 - Blocks of 4 output rows (2 pairs): pair cp -> PSUM bank (il) partitions
   64cp+32h+co = row y0+2cp+h. 2 banks per block, bufs=3 -> 6 of 8 banks.
 - Drains are full [128, 510] bias-adds psum->bf16, alternating ScalarE
   (img0) / VectorE (img1). Row block 127 is y0=506 (recomputes rows
   506/507) so all drains stay uniform; host takes rows 508/509 from it.
"""
import numpy as np
from ml_dtypes import bfloat16

import concourse.bass as bass
import concourse.tile as tile
from concourse import bacc, mybir
from concourse.bass_utils import run_bass_kernel_spmd
from contextlib import ExitStack

F32 = mybir.dt.float32
BF16 = mybir.dt.bfloat16

N_FULL, C, H, W = 16, 32, 512, 512
HO = WO = 510
N_CORES = 8
N_PER = N_FULL // N_CORES          # 2 images per core
N_STRIPS = H // 32                 # 16 strips of 32 input rows
N_BLOCKS = 128                     # 4-output-row blocks (block 127: y0=506)


def _block_y0(mb):
    return 4 * mb if mb < N_BLOCKS - 1 else 506


def _build():
    nc = bacc.Bacc("TRN2", target_bir_lowering=False, debug=False, num_devices=1)
    x_d = nc.dram_tensor("x", [N_STRIPS // 2, 128, 16384], BF16,
                         kind="ExternalInput").ap()
    w_d = nc.dram_tensor("w", [128, 384], BF16, kind="ExternalInput").ap()
    b_d = nc.dram_tensor("b", [128, 1], F32, kind="ExternalInput").ap()
    o_d = nc.dram_tensor("out", [N_BLOCKS // 2, 128, 4 * WO], BF16,
                         kind="ExternalOutput").ap()

    with tile.TileContext(nc) as tc, ExitStack() as ctx:
        const_pool = ctx.enter_context(tc.tile_pool(name="const", bufs=1))
        x_pool = ctx.enter_context(tc.tile_pool(name="xs", bufs=4))
        ps_pool = ctx.enter_context(tc.tile_pool(name="ps", bufs=4, space="PSUM"))
        o_pool = ctx.enter_context(tc.tile_pool(name="ob", bufs=3))

        wv = const_pool.tile([128, 384], BF16)
        nc.sync.dma_start(wv[:], w_d[:])
        bt = const_pool.tile([128, 1], F32)
        nc.sync.dma_start(bt[:], b_d[:])

        xtiles = {}
        obtiles = {}

        def load_pair(s2):
            xa = x_pool.tile([128, 16384], BF16, tag="x", name=f"xs_{s2}")
            nc.scalar.dma_start(xa[:], x_d[s2])
            xtiles[2 * s2] = (xa, 0)
            xtiles[2 * s2 + 1] = (xa, 8192)

        def emit_block(mb):
            y0 = _block_y0(mb)
            banks = {}
            for il in range(N_PER):
                banks[il] = ps_pool.tile([128, 512], F32, tag=f"ps{il}",
                                         name=f"ps{il}_{mb}")
            for step in range(6):
                w, kw = divmod(step, 3)
                for il in range(N_PER):
                    for cp in range(2):
                        rw = y0 + 2 * cp + 2 * w       # window rows rw, rw+1
                        st, lrw = divmod(rw, 32)
                        t = lrw // 2
                        xa, base = xtiles[st]
                        off = base + 512 * t + kw
                        nc.tensor.matmul(
                            banks[il][64 * cp:64 * cp + 64, 0:WO],
                            wv[64 * il:64 * il + 64,
                               64 * (3 * w + kw):64 * (3 * w + kw) + 64],
                            xa[64 * il:64 * il + 64, off:off + WO],
                            start=(step == 0), stop=(step == 5),
                            skip_group_check=True,
                            tile_position=(64 * il, 64 * cp),
                        )
            # four consecutive blocks share one [128, 2040] output tile per
            # image -> 4080B output descriptors and a quarter of the issues.
            pb, half = divmod(mb, 4)
            for il in range(N_PER):
                if half == 0:
                    obtiles[il] = o_pool.tile([128, 4 * WO], BF16,
                                              tag=f"ob{il}", name=f"ob{il}_{pb}")
                ob = obtiles[il]
                src = banks[il][0:128, 0:WO]
                dst = ob[:, half * WO:half * WO + WO]
                if il == 0:
                    nc.scalar.activation(
                        dst, src, mybir.ActivationFunctionType.Identity,
                        bias=bt[:])
                else:
                    nc.vector.tensor_scalar_add(dst, src, bt[:])
                if half == 3:
                    nc.sync.dma_start(o_d[2 * pb + il], ob[:])

        load_pair(0)
        load_pair(1)
        for s2 in range(N_STRIPS // 2):
            if s2 + 2 < N_STRIPS // 2:
                load_pair(s2 + 2)
            for mb in range(16 * s2, min(16 * s2 + 16, N_BLOCKS)):
                emit_block(mb)

    nc.compile()
    return nc


def _prep_inputs(x, weight, bias):
    """Host-side shard + relayout. Returns per-core in_maps."""
    x = np.asarray(x, dtype=np.float32)
    weight = np.asarray(weight, dtype=np.float32)
    bias = np.asarray(bias, dtype=np.float32)

    # x[2i+il, ci, 32(2*s2+sodd)+2t+q, w]
    #   -> xs[i, s2, 64*il+32*q+ci, 8192*sodd + 512*t + w]
    xr = x.reshape(N_CORES, N_PER, C, N_STRIPS // 2, 2, 16, 2, W)
    xr = xr.transpose(0, 3, 1, 6, 2, 4, 5, 7)   # core, s2, il, q, ci, sodd, t, w
    xs = np.ascontiguousarray(xr).reshape(N_CORES, N_STRIPS // 2, 128, 16384)
    xs = xs.astype(bfloat16)

    # wv[64il + 32q + ci, 64*(3w+kw) + 32h + co] = weight[co, ci, 2w+q-h, kw]
    # (zero when kh = 2w+q-h is outside [0, 3))
    wk = np.zeros((2, 32, 6, 2, 32), dtype=np.float32)  # q, ci, (w,kw), h, co
    for w in range(2):
        for kw in range(3):
            for q in range(2):
                for h in range(2):
                    kh = 2 * w + q - h
                    if 0 <= kh <= 2:
                        wk[q, :, 3 * w + kw, h, :] = weight[:, :, kh, kw].T
    wv = wk.transpose(0, 1, 2, 3, 4).reshape(64, 384)
    wv = np.tile(wv, (2, 1)).astype(bfloat16)
    bt = np.tile(bias, 4)[:, None].astype(np.float32)

    return [{"x": xs[i], "w": wv, "b": bt} for i in range(N_CORES)]


def _unpack_output(results):
    """results: list of 8 dicts with 'out' [64, 128, 2040] bf16."""
    dev = np.stack([r["out"] for r in results], axis=0)
    # [core, pb, il, cp, h, co, half, w]; row = 16pb + 4half + 2cp + h
    dev = dev.reshape(N_CORES, N_BLOCKS // 4, N_PER, 2, 2, C, 4, WO)
    full = dev.transpose(0, 2, 5, 1, 6, 3, 4, 7)
    full = full.reshape(N_CORES, N_PER, C, 512, WO)
    out = np.empty((N_FULL, C, HO, WO), dtype=np.float32)
    o = out.reshape(N_CORES, N_PER, C, HO, WO)
    o[:, :, :, :508, :] = full[:, :, :, :508, :]
    # block 127 (pb=31, half=3) holds rows 506 + 2cp + h -> composed rows
    # 510/511 are true rows 508/509
    o[:, :, :, 508:510, :] = full[:, :, :, 510:512, :]
    return out


_NC = None


def kernel(x, weight, bias):
    global _NC
    if _NC is None:
        _NC = _build()
    in_maps = _prep_inputs(x, weight, bias)
    res = run_bass_kernel_spmd(_NC, in_maps, core_ids=list(range(N_CORES)))
    return _unpack_output(res.results)
